# revision 3
# baseline (speedup 1.0000x reference)
"""CIN (Compressed Interaction Network) kernel for Trainium2, 8-core data parallel.

Reference computation (per batch element b, position d):
  hidden = x                                  # (39 fields)
  layer i: z[(m,n)] = x[m] * hidden[n]        # outer product over fields
           cur[o]   = relu(sum_c z[c] W_i[c,o] + b_i[o])   # 200 outs
           hidden, direct = cur[:100], cur[100:]  (layers 0,1);  direct = cur (layer 2)
  out[b, j] = sum_d concat(directs)[j, d]     # (2048, 400)

Strategy: batch sharded across 8 cores (256 batch each, rows = b*32+d -> 8192).
Channel-major layout everywhere: X (39p, rows), H (100p, rows), all bf16.
z built on VectorE as XR (X broadcast across partitions via DMA from DRAM)
times H (free-dim broadcast).  TensorE contracts z blocks against the
(statically reordered) weights with PSUM accumulation; ScalarE applies
bias+ReLU+cast; VectorE reduces over d into the output accumulator.
"""

import sys

sys.path.insert(0, '/opt/trn_rl_repo')

import numpy as np
import ml_dtypes

import concourse.bacc as bacc
import concourse.mybir as mybir
import concourse.tile as tile
from concourse import bass_utils

BF16 = ml_dtypes.bfloat16

NCORES = 8
B = 2048
BC = B // NCORES          # 256 batch per core
D = 32
ROWS = BC * D             # 8192
F0 = 39
FK = 100
O = 200
RT = 512                  # rows per tile
NRT = ROWS // RT          # 16
BPT = RT // D             # batches per row tile = 16
G0 = 3                    # layer-0: 3 m-groups per K-block
P0 = G0 * F0              # 117
KB0 = F0 // G0            # 13
ZG = 13                   # layer-1/2: m-blocks per z-group
NZG = F0 // ZG            # 3 groups

_cached = {}


def _emit(tc, outs, ins):
    nc = tc.nc
    x_d = ins['x_t']
    w0_d = ins['w0']
    w1_d = ins['w1']
    w2_d = ins['w2']
    b_d = ins['bias']
    out_d = outs['out']

    bf = mybir.dt.bfloat16
    f32 = mybir.dt.float32
    mult = mybir.AluOpType.mult
    add = mybir.AluOpType.add
    relu = mybir.ActivationFunctionType.Relu
    X = mybir.AxisListType.X

    import contextlib
    ctx = contextlib.ExitStack()
    with ctx:
        const = ctx.enter_context(tc.tile_pool(name="const", bufs=1))
        accp = ctx.enter_context(tc.tile_pool(name="acc", bufs=1))
        xrp = ctx.enter_context(tc.tile_pool(name="xr", bufs=1))
        xr0p = ctx.enter_context(tc.tile_pool(name="xr0", bufs=2))
        xt0p = ctx.enter_context(tc.tile_pool(name="xt0", bufs=2))
        z0p = ctx.enter_context(tc.tile_pool(name="z0", bufs=2))
        zp = ctx.enter_context(tc.tile_pool(name="z", bufs=2))
        hp = ctx.enter_context(tc.tile_pool(name="h", bufs=3))
        dp = ctx.enter_context(tc.tile_pool(name="d", bufs=2))
        psum = ctx.enter_context(tc.tile_pool(name="ps", bufs=6, space="PSUM"))

        # resident weights / bias
        w0_sb = const.tile([P0, KB0, O], bf, tag="w0")
        nc.sync.dma_start(w0_sb, w0_d)
        w1_sb = const.tile([FK, F0, O], bf, tag="w1")
        nc.sync.dma_start(w1_sb, w1_d)
        w2_sb = const.tile([FK, F0, O], bf, tag="w2")
        nc.sync.dma_start(w2_sb, w2_d)
        b_sb = const.tile([FK, 6], f32, tag="bias")
        nc.sync.dma_start(b_sb, b_d)

        # output accumulators (j-group on partitions, batch on free)
        acc = [accp.tile([FK, BC], f32, tag=f"acc{i}", name=f"acc{i}") for i in range(4)]

        for rt in range(NRT):
            rs = slice(rt * RT, (rt + 1) * RT)
            bs = slice(rt * BPT, (rt + 1) * BPT)

            # ---- layer-0 inputs: XR0[p=(dm,n),kb,r] = x[3kb+dm, r]; XT0[p] = x[n, r]
            xr0 = xr0p.tile([P0, KB0, RT], bf, tag="xr0")
            src3 = x_d[:, rs].rearrange("(kb dm) r -> dm kb r", dm=G0)
            for dmi in range(G0):
                nc.sync.dma_start(
                    xr0[dmi * F0:(dmi + 1) * F0, :, :],
                    src3[dmi][None, :, :].to_broadcast((F0, KB0, RT)))
            xt0 = xt0p.tile([P0, RT], bf, tag="xt0")
            nc.sync.dma_start(xt0, x_d[:, rs][None, :, :].to_broadcast((G0, F0, RT)))

            # XR for layers 1/2: (FK, F0, RT) = x broadcast over 100 partitions
            xr = xrp.tile([FK, F0, RT], bf, tag="xr")
            nc.sync.dma_start(xr, x_d[:, rs][None, :, :].to_broadcast((FK, F0, RT)))

            # ---- layer 0
            z0 = z0p.tile([P0, KB0, RT], bf, tag="z0")
            nc.vector.tensor_tensor(
                z0, xr0, xt0[:, None, :].to_broadcast((P0, KB0, RT)), mult)
            ps0 = [psum.tile([FK, RT], f32, tag="ps", name=f"ps0_{rt}_{t}") for t in range(2)]
            for kb in range(KB0):
                st = kb == 0
                sp = kb == KB0 - 1
                for t in range(2):
                    nc.tensor.matmul(ps0[t], w0_sb[:, kb, t * FK:(t + 1) * FK],
                                     z0[:, kb, :], start=st, stop=sp)
            h1 = hp.tile([FK, RT], bf, tag="h")
            nc.scalar.activation(h1, ps0[0], relu, bias=b_sb[:, 0:1])
            d0 = dp.tile([FK, RT], f32, tag="d")
            nc.scalar.activation(d0, ps0[1], relu, bias=b_sb[:, 1:2])
            nc.vector.tensor_reduce(
                acc[0][:, bs], d0.rearrange("o (g f) -> o g f", f=D), X, add)

            # ---- layers 1, 2
            hprev = h1
            for li, (w_sb, bcol) in enumerate(((w1_sb, 2), (w2_sb, 4))):
                ps = [psum.tile([FK, RT], f32, tag="ps", name=f"psl{li}_{rt}_{t}") for t in range(2)]
                for g in range(NZG):
                    zt = zp.tile([FK, ZG, RT], bf, tag="z")
                    nc.vector.tensor_tensor(
                        zt, xr[:, g * ZG:(g + 1) * ZG, :],
                        hprev[:, None, :].to_broadcast((FK, ZG, RT)), mult)
                    for j in range(ZG):
                        m = g * ZG + j
                        st = m == 0
                        sp = m == F0 - 1
                        for t in range(2):
                            nc.tensor.matmul(ps[t], w_sb[:, m, t * FK:(t + 1) * FK],
                                             zt[:, j, :], start=st, stop=sp)
                if li == 0:
                    h2 = hp.tile([FK, RT], bf, tag="h")
                    nc.scalar.activation(h2, ps[0], relu, bias=b_sb[:, bcol:bcol + 1])
                    d1 = dp.tile([FK, RT], f32, tag="d")
                    nc.scalar.activation(d1, ps[1], relu, bias=b_sb[:, bcol + 1:bcol + 2])
                    nc.vector.tensor_reduce(
                        acc[1][:, bs], d1.rearrange("o (g f) -> o g f", f=D), X, add)
                    hprev = h2
                else:
                    for t in range(2):
                        d2 = dp.tile([FK, RT], f32, tag="d")
                        nc.scalar.activation(d2, ps[t], relu,
                                             bias=b_sb[:, bcol + t:bcol + t + 1])
                        nc.vector.tensor_reduce(
                            acc[2 + t][:, bs],
                            d2.rearrange("o (g f) -> o g f", f=D), X, add)

        for i in range(4):
            nc.sync.dma_start(out_d[i * FK:(i + 1) * FK, :], acc[i])


def _prep_weights(W0, W1, W2, b0, b1, b2):
    w0 = np.ascontiguousarray(
        W0.reshape(KB0, P0, O).transpose(1, 0, 2)).astype(BF16)
    w1 = np.ascontiguousarray(
        W1.reshape(F0, FK, O).transpose(1, 0, 2)).astype(BF16)
    w2 = np.ascontiguousarray(
        W2.reshape(F0, FK, O).transpose(1, 0, 2)).astype(BF16)
    bias = np.ascontiguousarray(
        np.stack([b0, b1, b2]).reshape(3, 2, FK).transpose(2, 0, 1).reshape(FK, 6)
    ).astype(np.float32)
    return w0, w1, w2, bias


def _prep_x_shard(x, c):
    xs = x[c * BC:(c + 1) * BC]                      # (BC, 39, 32)
    x_t = np.ascontiguousarray(xs.transpose(1, 0, 2).reshape(F0, ROWS))
    return x_t.astype(BF16)


def _build():
    if 'nc' in _cached:
        return _cached['nc']
    nc = bacc.Bacc("TRN2", target_bir_lowering=False, debug=False,
                   enable_asserts=False, num_devices=NCORES)
    ins = {
        'x_t': nc.dram_tensor("x_t", (F0, ROWS), mybir.dt.bfloat16,
                              kind="ExternalInput").ap(),
        'w0': nc.dram_tensor("w0", (P0, KB0, O), mybir.dt.bfloat16,
                             kind="ExternalInput").ap(),
        'w1': nc.dram_tensor("w1", (FK, F0, O), mybir.dt.bfloat16,
                             kind="ExternalInput").ap(),
        'w2': nc.dram_tensor("w2", (FK, F0, O), mybir.dt.bfloat16,
                             kind="ExternalInput").ap(),
        'bias': nc.dram_tensor("bias", (FK, 6), mybir.dt.float32,
                               kind="ExternalInput").ap(),
    }
    outs = {
        'out': nc.dram_tensor("out", (4 * FK, BC), mybir.dt.float32,
                              kind="ExternalOutput").ap(),
    }
    with tile.TileContext(nc, trace_sim=False) as tc:
        _emit(tc, outs, ins)
    nc.compile()
    _cached['nc'] = nc
    return nc


def kernel(x, W0, W1, W2, b0, b1, b2):
    nc = _build()
    w0, w1, w2, bias = _prep_weights(
        np.asarray(W0, np.float32), np.asarray(W1, np.float32),
        np.asarray(W2, np.float32), np.asarray(b0, np.float32),
        np.asarray(b1, np.float32), np.asarray(b2, np.float32))
    x = np.asarray(x, np.float32)
    in_maps = []
    for c in range(NCORES):
        in_maps.append({
            'x_t': _prep_x_shard(x, c),
            'w0': w0, 'w1': w1, 'w2': w2, 'bias': bias,
        })
    res = bass_utils.run_bass_kernel_spmd(
        nc, in_maps, core_ids=list(range(NCORES)))
    out = np.empty((B, 4 * FK), np.float32)
    for c in range(NCORES):
        out[c * BC:(c + 1) * BC, :] = res.results[c]['out'].T
    return out


# revision 5
# speedup vs baseline: 1.1374x; 1.1374x over previous
"""CIN (Compressed Interaction Network) kernel for Trainium2, 8-core data parallel.

Reference computation (per batch element b, position d):
  hidden = x                                  # (39 fields)
  layer i: z[(m,n)] = x[m] * hidden[n]        # outer product over fields
           cur[o]   = relu(sum_c z[c] W_i[c,o] + b_i[o])   # 200 outs
           hidden, direct = cur[:100], cur[100:]  (layers 0,1);  direct = cur (layer 2)
  out[b, j] = sum_d concat(directs)[j, d]     # (2048, 400)

Strategy: batch sharded across 8 cores (256 batch each, rows = b*32+d -> 8192).
Channel-major layout everywhere: X (39p, rows), H (100p, rows), all bf16.
z built on VectorE as XR (X broadcast across partitions via DMA from DRAM)
times H (free-dim broadcast).  TensorE contracts z blocks against the
(statically reordered) weights with PSUM accumulation; ScalarE applies
bias+ReLU+cast; VectorE reduces over d into the output accumulator.
"""

import sys

sys.path.insert(0, '/opt/trn_rl_repo')

import numpy as np
import ml_dtypes

import concourse.bacc as bacc
import concourse.mybir as mybir
import concourse.tile as tile
from concourse import bass_utils

BF16 = ml_dtypes.bfloat16

NCORES = 8
B = 2048
BC = B // NCORES          # 256 batch per core
D = 32
ROWS = BC * D             # 8192
F0 = 39
FK = 100
O = 200
RT = 512                  # rows per tile
NRT = ROWS // RT          # 16
BPT = RT // D             # batches per row tile = 16
G0 = 3                    # layer-0: 3 m-groups per K-block
P0 = G0 * F0              # 117
KB0 = F0 // G0            # 13
ZG = 13                   # layer-1/2: m-blocks per z-group
NZG = F0 // ZG            # 3 groups

_cached = {}


def _emit(tc, outs, ins):
    nc = tc.nc
    x_d = ins['x_t']
    w0_d = ins['w0']
    w1_d = ins['w1']
    w2_d = ins['w2']
    b_d = ins['bias']
    out_d = outs['out']

    bf = mybir.dt.bfloat16
    f32 = mybir.dt.float32
    mult = mybir.AluOpType.mult
    add = mybir.AluOpType.add
    relu = mybir.ActivationFunctionType.Relu
    X = mybir.AxisListType.X

    import contextlib
    ctx = contextlib.ExitStack()
    with ctx:
        const = ctx.enter_context(tc.tile_pool(name="const", bufs=1))
        accp = ctx.enter_context(tc.tile_pool(name="acc", bufs=1))
        xrp = ctx.enter_context(tc.tile_pool(name="xr", bufs=1))
        xr0p = ctx.enter_context(tc.tile_pool(name="xr0", bufs=2))
        xt0p = ctx.enter_context(tc.tile_pool(name="xt0", bufs=2))
        z0p = ctx.enter_context(tc.tile_pool(name="z0", bufs=2))
        zp = ctx.enter_context(tc.tile_pool(name="z", bufs=2))
        hp = ctx.enter_context(tc.tile_pool(name="h", bufs=3))
        dp = ctx.enter_context(tc.tile_pool(name="d", bufs=2))
        psum = ctx.enter_context(tc.tile_pool(name="ps", bufs=6, space="PSUM"))

        # resident weights / bias
        w0_sb = const.tile([P0, KB0, O], bf, tag="w0")
        nc.sync.dma_start(w0_sb, w0_d)
        w1_sb = const.tile([FK, F0, O], bf, tag="w1")
        nc.sync.dma_start(w1_sb, w1_d)
        w2_sb = const.tile([FK, F0, O], bf, tag="w2")
        nc.sync.dma_start(w2_sb, w2_d)
        b_sb = const.tile([FK, 6], f32, tag="bias")
        nc.sync.dma_start(b_sb, b_d)

        # output accumulators (j-group on partitions, batch on free)
        acc = [accp.tile([FK, BC], f32, tag=f"acc{i}", name=f"acc{i}") for i in range(4)]

        for rt in range(NRT):
            bs = slice(rt * BPT, (rt + 1) * BPT)
            xslab = x_d[rt]  # (F0, RT), contiguous

            # ---- layer-0 inputs: XR0[p=(dm,n),kb,r] = x[3kb+dm, r]; XT0[p] = x[n, r]
            xr0 = xr0p.tile([P0, KB0, RT], bf, tag="xr0")
            src3 = xslab.rearrange("(kb dm) r -> dm kb r", dm=G0)
            for dmi in range(G0):
                nc.sync.dma_start(
                    xr0[dmi * F0:(dmi + 1) * F0, :, :],
                    src3[dmi][None, :, :].to_broadcast((F0, KB0, RT)))
            xt0 = xt0p.tile([P0, RT], bf, tag="xt0")
            nc.sync.dma_start(xt0, xslab[None, :, :].to_broadcast((G0, F0, RT)))

            # XR for layers 1/2: (FK, F0, RT) = x broadcast over 100 partitions
            xr = xrp.tile([FK, F0, RT], bf, tag="xr")
            nc.sync.dma_start(xr, xslab[None, :, :].to_broadcast((FK, F0, RT)))

            # ---- layer 0
            z0 = z0p.tile([P0, KB0, RT], bf, tag="z0")
            nc.gpsimd.tensor_tensor(
                z0, xr0, xt0[:, None, :].to_broadcast((P0, KB0, RT)), mult)
            ps0 = [psum.tile([FK, RT], f32, tag="ps", name=f"ps0_{rt}_{t}") for t in range(2)]
            for kb in range(KB0):
                st = kb == 0
                sp = kb == KB0 - 1
                for t in range(2):
                    nc.tensor.matmul(ps0[t], w0_sb[:, kb, t * FK:(t + 1) * FK],
                                     z0[:, kb, :], start=st, stop=sp)
            h1 = hp.tile([FK, RT], bf, tag="h")
            nc.scalar.activation(h1, ps0[0], relu, bias=b_sb[:, 0:1])
            d0 = dp.tile([FK, RT], f32, tag="d")
            nc.scalar.activation(d0, ps0[1], relu, bias=b_sb[:, 1:2])
            nc.vector.tensor_reduce(
                acc[0][:, bs], d0.rearrange("o (g f) -> o g f", f=D), X, add)

            # ---- layers 1, 2
            hprev = h1
            for li, (w_sb, bcol) in enumerate(((w1_sb, 2), (w2_sb, 4))):
                ps = [psum.tile([FK, RT], f32, tag="ps", name=f"psl{li}_{rt}_{t}") for t in range(2)]
                for g in range(NZG):
                    zt = zp.tile([FK, ZG, RT], bf, tag="z")
                    nc.vector.tensor_tensor(
                        zt, xr[:, g * ZG:(g + 1) * ZG, :],
                        hprev[:, None, :].to_broadcast((FK, ZG, RT)), mult)
                    for j in range(ZG):
                        m = g * ZG + j
                        st = m == 0
                        sp = m == F0 - 1
                        for t in range(2):
                            nc.tensor.matmul(ps[t], w_sb[:, m, t * FK:(t + 1) * FK],
                                             zt[:, j, :], start=st, stop=sp)
                if li == 0:
                    h2 = hp.tile([FK, RT], bf, tag="h")
                    nc.scalar.activation(h2, ps[0], relu, bias=b_sb[:, bcol:bcol + 1])
                    d1 = dp.tile([FK, RT], f32, tag="d")
                    nc.scalar.activation(d1, ps[1], relu, bias=b_sb[:, bcol + 1:bcol + 2])
                    nc.vector.tensor_reduce(
                        acc[1][:, bs], d1.rearrange("o (g f) -> o g f", f=D), X, add)
                    hprev = h2
                else:
                    for t in range(2):
                        d2 = dp.tile([FK, RT], f32, tag="d")
                        nc.scalar.activation(d2, ps[t], relu,
                                             bias=b_sb[:, bcol + t:bcol + t + 1])
                        nc.vector.tensor_reduce(
                            acc[2 + t][:, bs],
                            d2.rearrange("o (g f) -> o g f", f=D), X, add)

        for i in range(4):
            nc.sync.dma_start(out_d[i * FK:(i + 1) * FK, :], acc[i])


def _prep_weights(W0, W1, W2, b0, b1, b2):
    w0 = np.ascontiguousarray(
        W0.reshape(KB0, P0, O).transpose(1, 0, 2)).astype(BF16)
    w1 = np.ascontiguousarray(
        W1.reshape(F0, FK, O).transpose(1, 0, 2)).astype(BF16)
    w2 = np.ascontiguousarray(
        W2.reshape(F0, FK, O).transpose(1, 0, 2)).astype(BF16)
    bias = np.ascontiguousarray(
        np.stack([b0, b1, b2]).reshape(3, 2, FK).transpose(2, 0, 1).reshape(FK, 6)
    ).astype(np.float32)
    return w0, w1, w2, bias


def _prep_x_shard(x, c):
    xs = x[c * BC:(c + 1) * BC]                      # (BC, 39, 32)
    x_t = xs.transpose(1, 0, 2).reshape(F0, NRT, RT).transpose(1, 0, 2)
    return np.ascontiguousarray(x_t).astype(BF16)


def _build():
    if 'nc' in _cached:
        return _cached['nc']
    nc = bacc.Bacc("TRN2", target_bir_lowering=False, debug=False,
                   enable_asserts=False, num_devices=NCORES)
    ins = {
        'x_t': nc.dram_tensor("x_t", (NRT, F0, RT), mybir.dt.bfloat16,
                              kind="ExternalInput").ap(),
        'w0': nc.dram_tensor("w0", (P0, KB0, O), mybir.dt.bfloat16,
                             kind="ExternalInput").ap(),
        'w1': nc.dram_tensor("w1", (FK, F0, O), mybir.dt.bfloat16,
                             kind="ExternalInput").ap(),
        'w2': nc.dram_tensor("w2", (FK, F0, O), mybir.dt.bfloat16,
                             kind="ExternalInput").ap(),
        'bias': nc.dram_tensor("bias", (FK, 6), mybir.dt.float32,
                               kind="ExternalInput").ap(),
    }
    outs = {
        'out': nc.dram_tensor("out", (4 * FK, BC), mybir.dt.float32,
                              kind="ExternalOutput").ap(),
    }
    with tile.TileContext(nc, trace_sim=False) as tc:
        _emit(tc, outs, ins)
    nc.compile()
    _cached['nc'] = nc
    return nc


def kernel(x, W0, W1, W2, b0, b1, b2):
    nc = _build()
    w0, w1, w2, bias = _prep_weights(
        np.asarray(W0, np.float32), np.asarray(W1, np.float32),
        np.asarray(W2, np.float32), np.asarray(b0, np.float32),
        np.asarray(b1, np.float32), np.asarray(b2, np.float32))
    x = np.asarray(x, np.float32)
    in_maps = []
    for c in range(NCORES):
        in_maps.append({
            'x_t': _prep_x_shard(x, c),
            'w0': w0, 'w1': w1, 'w2': w2, 'bias': bias,
        })
    res = bass_utils.run_bass_kernel_spmd(
        nc, in_maps, core_ids=list(range(NCORES)))
    out = np.empty((B, 4 * FK), np.float32)
    for c in range(NCORES):
        out[c * BC:(c + 1) * BC, :] = res.results[c]['out'].T
    return out


# revision 6
# speedup vs baseline: 1.1403x; 1.0026x over previous
"""CIN (Compressed Interaction Network) kernel for Trainium2, 8-core data parallel.

Reference computation (per batch element b, position d):
  hidden = x                                  # (39 fields)
  layer i: z[(m,n)] = x[m] * hidden[n]        # outer product over fields
           cur[o]   = relu(sum_c z[c] W_i[c,o] + b_i[o])   # 200 outs
           hidden, direct = cur[:100], cur[100:]  (layers 0,1);  direct = cur (layer 2)
  out[b, j] = sum_d concat(directs)[j, d]     # (2048, 400)

Strategy: batch sharded across 8 cores (256 batch each, rows = b*32+d -> 8192).
Channel-major layout everywhere: X (39p, rows), H (100p, rows), all bf16.
z built on VectorE as XR (X broadcast across partitions via DMA from DRAM)
times H (free-dim broadcast).  TensorE contracts z blocks against the
(statically reordered) weights with PSUM accumulation; ScalarE applies
bias+ReLU+cast; VectorE reduces over d into the output accumulator.
"""

import sys

sys.path.insert(0, '/opt/trn_rl_repo')

import numpy as np
import ml_dtypes

import concourse.bacc as bacc
import concourse.mybir as mybir
import concourse.tile as tile
from concourse import bass_utils

BF16 = ml_dtypes.bfloat16

NCORES = 8
B = 2048
BC = B // NCORES          # 256 batch per core
D = 32
ROWS = BC * D             # 8192
F0 = 39
FK = 100
O = 200
RT = 512                  # rows per tile
NRT = ROWS // RT          # 16
BPT = RT // D             # batches per row tile = 16
G0 = 3                    # layer-0: 3 m-groups per K-block
P0 = G0 * F0              # 117
KB0 = F0 // G0            # 13
ZG = 13                   # layer-1/2: m-blocks per z-group
NZG = F0 // ZG            # 3 groups

_cached = {}


def _emit(tc, outs, ins):
    nc = tc.nc
    x_d = ins['x_t']
    x0_d = ins['x0']
    w0_d = ins['w0']
    w1_d = ins['w1']
    w2_d = ins['w2']
    b_d = ins['bias']
    out_d = outs['out']

    bf = mybir.dt.bfloat16
    f32 = mybir.dt.float32
    mult = mybir.AluOpType.mult
    add = mybir.AluOpType.add
    relu = mybir.ActivationFunctionType.Relu
    X = mybir.AxisListType.X

    import contextlib
    ctx = contextlib.ExitStack()
    with ctx:
        const = ctx.enter_context(tc.tile_pool(name="const", bufs=1))
        accp = ctx.enter_context(tc.tile_pool(name="acc", bufs=1))
        xrp = ctx.enter_context(tc.tile_pool(name="xr", bufs=1))
        xr0p = ctx.enter_context(tc.tile_pool(name="xr0", bufs=2))
        xt0p = ctx.enter_context(tc.tile_pool(name="xt0", bufs=2))
        z0p = ctx.enter_context(tc.tile_pool(name="z0", bufs=2))
        zp = ctx.enter_context(tc.tile_pool(name="z", bufs=2))
        hp = ctx.enter_context(tc.tile_pool(name="h", bufs=3))
        dp = ctx.enter_context(tc.tile_pool(name="d", bufs=2))
        psum = ctx.enter_context(tc.tile_pool(name="ps", bufs=6, space="PSUM"))

        # resident weights / bias
        w0_sb = const.tile([P0, KB0, O], bf, tag="w0")
        nc.sync.dma_start(w0_sb, w0_d)
        w1_sb = const.tile([FK, F0, O], bf, tag="w1")
        nc.sync.dma_start(w1_sb, w1_d)
        w2_sb = const.tile([FK, F0, O], bf, tag="w2")
        nc.sync.dma_start(w2_sb, w2_d)
        b_sb = const.tile([FK, 6], f32, tag="bias")
        nc.sync.dma_start(b_sb, b_d)

        # output accumulators (j-group on partitions, batch on free)
        acc = [accp.tile([FK, BC], f32, tag=f"acc{i}", name=f"acc{i}") for i in range(4)]

        for rt in range(NRT):
            bs = slice(rt * BPT, (rt + 1) * BPT)
            xslab = x_d[rt]  # (F0, RT), contiguous

            # ---- layer-0 inputs: XR0[p=(dm,n),kb,r] = x[3kb+dm, r]; XT0[p] = x[n, r]
            xr0 = xr0p.tile([P0, KB0, RT], bf, tag="xr0")
            for dmi in range(G0):
                nc.sync.dma_start(
                    xr0[dmi * F0:(dmi + 1) * F0, :, :],
                    x0_d[rt, dmi][None, :, :].to_broadcast((F0, KB0, RT)))
            xt0 = xt0p.tile([P0, RT], bf, tag="xt0")
            nc.sync.dma_start(xt0, xslab[None, :, :].to_broadcast((G0, F0, RT)))

            # XR for layers 1/2: (FK, F0, RT) = x broadcast over 100 partitions
            xr = xrp.tile([FK, F0, RT], bf, tag="xr")
            nc.sync.dma_start(xr, xslab[None, :, :].to_broadcast((FK, F0, RT)))

            # ---- layer 0
            z0 = z0p.tile([P0, KB0, RT], bf, tag="z0")
            nc.gpsimd.tensor_tensor(
                z0, xr0, xt0[:, None, :].to_broadcast((P0, KB0, RT)), mult)
            ps0 = [psum.tile([FK, RT], f32, tag="ps", name=f"ps0_{rt}_{t}") for t in range(2)]
            for kb in range(KB0):
                st = kb == 0
                sp = kb == KB0 - 1
                for t in range(2):
                    nc.tensor.matmul(ps0[t], w0_sb[:, kb, t * FK:(t + 1) * FK],
                                     z0[:, kb, :], start=st, stop=sp)
            h1 = hp.tile([FK, RT], bf, tag="h")
            nc.scalar.activation(h1, ps0[0], relu, bias=b_sb[:, 0:1])
            d0 = dp.tile([FK, RT], f32, tag="d")
            nc.scalar.activation(d0, ps0[1], relu, bias=b_sb[:, 1:2])
            nc.vector.tensor_reduce(
                acc[0][:, bs], d0.rearrange("o (g f) -> o g f", f=D), X, add)

            # ---- layers 1, 2
            hprev = h1
            for li, (w_sb, bcol) in enumerate(((w1_sb, 2), (w2_sb, 4))):
                ps = [psum.tile([FK, RT], f32, tag="ps", name=f"psl{li}_{rt}_{t}") for t in range(2)]
                for g in range(NZG):
                    zt = zp.tile([FK, ZG, RT], bf, tag="z")
                    nc.vector.tensor_tensor(
                        zt, xr[:, g * ZG:(g + 1) * ZG, :],
                        hprev[:, None, :].to_broadcast((FK, ZG, RT)), mult)
                    for j in range(ZG):
                        m = g * ZG + j
                        st = m == 0
                        sp = m == F0 - 1
                        for t in range(2):
                            nc.tensor.matmul(ps[t], w_sb[:, m, t * FK:(t + 1) * FK],
                                             zt[:, j, :], start=st, stop=sp)
                if li == 0:
                    h2 = hp.tile([FK, RT], bf, tag="h")
                    nc.scalar.activation(h2, ps[0], relu, bias=b_sb[:, bcol:bcol + 1])
                    d1 = dp.tile([FK, RT], f32, tag="d")
                    nc.scalar.activation(d1, ps[1], relu, bias=b_sb[:, bcol + 1:bcol + 2])
                    nc.vector.tensor_reduce(
                        acc[1][:, bs], d1.rearrange("o (g f) -> o g f", f=D), X, add)
                    hprev = h2
                else:
                    for t in range(2):
                        d2 = dp.tile([FK, RT], f32, tag="d")
                        nc.scalar.activation(d2, ps[t], relu,
                                             bias=b_sb[:, bcol + t:bcol + t + 1])
                        nc.vector.tensor_reduce(
                            acc[2 + t][:, bs],
                            d2.rearrange("o (g f) -> o g f", f=D), X, add)

        for i in range(4):
            nc.sync.dma_start(out_d[i * FK:(i + 1) * FK, :], acc[i])


def _prep_weights(W0, W1, W2, b0, b1, b2):
    w0 = np.ascontiguousarray(
        W0.reshape(KB0, P0, O).transpose(1, 0, 2)).astype(BF16)
    w1 = np.ascontiguousarray(
        W1.reshape(F0, FK, O).transpose(1, 0, 2)).astype(BF16)
    w2 = np.ascontiguousarray(
        W2.reshape(F0, FK, O).transpose(1, 0, 2)).astype(BF16)
    bias = np.ascontiguousarray(
        np.stack([b0, b1, b2]).reshape(3, 2, FK).transpose(2, 0, 1).reshape(FK, 6)
    ).astype(np.float32)
    return w0, w1, w2, bias


def _prep_x_shard(x, c):
    xs = x[c * BC:(c + 1) * BC]                      # (BC, 39, 32)
    xt = xs.transpose(1, 0, 2).reshape(F0, ROWS)          # (39, 8192)
    x_t = np.ascontiguousarray(
        xt.reshape(F0, NRT, RT).transpose(1, 0, 2)).astype(BF16)
    x_perm = np.ascontiguousarray(
        xt.reshape(KB0, G0, NRT, RT).transpose(2, 1, 0, 3)).astype(BF16)
    return x_t, x_perm


def _build():
    if 'nc' in _cached:
        return _cached['nc']
    nc = bacc.Bacc("TRN2", target_bir_lowering=False, debug=False,
                   enable_asserts=False, num_devices=NCORES)
    ins = {
        'x_t': nc.dram_tensor("x_t", (NRT, F0, RT), mybir.dt.bfloat16,
                              kind="ExternalInput").ap(),
        'x0': nc.dram_tensor("x0", (NRT, G0, KB0, RT), mybir.dt.bfloat16,
                             kind="ExternalInput").ap(),
        'w0': nc.dram_tensor("w0", (P0, KB0, O), mybir.dt.bfloat16,
                             kind="ExternalInput").ap(),
        'w1': nc.dram_tensor("w1", (FK, F0, O), mybir.dt.bfloat16,
                             kind="ExternalInput").ap(),
        'w2': nc.dram_tensor("w2", (FK, F0, O), mybir.dt.bfloat16,
                             kind="ExternalInput").ap(),
        'bias': nc.dram_tensor("bias", (FK, 6), mybir.dt.float32,
                               kind="ExternalInput").ap(),
    }
    outs = {
        'out': nc.dram_tensor("out", (4 * FK, BC), mybir.dt.float32,
                              kind="ExternalOutput").ap(),
    }
    with tile.TileContext(nc, trace_sim=False) as tc:
        _emit(tc, outs, ins)
    nc.compile()
    _cached['nc'] = nc
    return nc


def kernel(x, W0, W1, W2, b0, b1, b2):
    nc = _build()
    w0, w1, w2, bias = _prep_weights(
        np.asarray(W0, np.float32), np.asarray(W1, np.float32),
        np.asarray(W2, np.float32), np.asarray(b0, np.float32),
        np.asarray(b1, np.float32), np.asarray(b2, np.float32))
    x = np.asarray(x, np.float32)
    in_maps = []
    for c in range(NCORES):
        x_t, x_perm = _prep_x_shard(x, c)
        in_maps.append({
            'x_t': x_t, 'x0': x_perm,
            'w0': w0, 'w1': w1, 'w2': w2, 'bias': bias,
        })
    res = bass_utils.run_bass_kernel_spmd(
        nc, in_maps, core_ids=list(range(NCORES)))
    out = np.empty((B, 4 * FK), np.float32)
    for c in range(NCORES):
        out[c * BC:(c + 1) * BC, :] = res.results[c]['out'].T
    return out


# revision 8
# speedup vs baseline: 1.4378x; 1.2609x over previous
"""CIN (Compressed Interaction Network) kernel for Trainium2, 8-core data parallel.

Reference computation (per batch element b, position d):
  hidden = x                                  # (39 fields)
  layer i: z[(m,n)] = x[m] * hidden[n]        # outer product over fields
           cur[o]   = relu(sum_c z[c] W_i[c,o] + b_i[o])   # 200 outs
           hidden, direct = cur[:100], cur[100:]  (layers 0,1);  direct = cur (layer 2)
  out[b, j] = sum_d concat(directs)[j, d]     # (2048, 400)

Strategy: batch sharded across 8 cores (256 batch each, rows = b*32+d -> 8192).
Channel-major layout everywhere: X (39p, rows), H (100p, rows), all bf16.
z built on VectorE as XR (X broadcast across partitions via DMA from DRAM)
times H (free-dim broadcast).  TensorE contracts z blocks against the
(statically reordered) weights with PSUM accumulation; ScalarE applies
bias+ReLU+cast; VectorE reduces over d into the output accumulator.
"""

import sys

sys.path.insert(0, '/opt/trn_rl_repo')

import numpy as np
import ml_dtypes

import concourse.bacc as bacc
import concourse.mybir as mybir
import concourse.tile as tile
from concourse import bass_utils

BF16 = ml_dtypes.bfloat16

NCORES = 8
B = 2048
BC = B // NCORES          # 256 batch per core
D = 32
ROWS = BC * D             # 8192
F0 = 39
FK = 100
O = 200
RT = 512                  # rows per tile
NRT = ROWS // RT          # 16
BPT = RT // D             # batches per row tile = 16
G0 = 3                    # layer-0: 3 m-groups per K-block
P0 = G0 * F0              # 117
KB0 = F0 // G0            # 13
ZGS = [5, 12, 11, 11]     # layer-1/2: z-group sizes (first small to refill PE fast)
ZGMAX = max(ZGS)

_cached = {}


def _emit(tc, outs, ins):
    nc = tc.nc
    x_d = ins['x_t']
    x0_d = ins['x0']
    w0_d = ins['w0']
    w1_d = ins['w1']
    w2_d = ins['w2']
    b_d = ins['bias']
    out_d = outs['out']

    bf = mybir.dt.bfloat16
    f32 = mybir.dt.float32
    mult = mybir.AluOpType.mult
    add = mybir.AluOpType.add
    relu = mybir.ActivationFunctionType.Relu
    X = mybir.AxisListType.X

    import contextlib
    ctx = contextlib.ExitStack()
    with ctx:
        const = ctx.enter_context(tc.tile_pool(name="const", bufs=1))
        accp = ctx.enter_context(tc.tile_pool(name="acc", bufs=1))
        xrp = ctx.enter_context(tc.tile_pool(name="xr", bufs=2))
        xr0p = ctx.enter_context(tc.tile_pool(name="xr0", bufs=1))
        xt0p = ctx.enter_context(tc.tile_pool(name="xt0", bufs=1))
        z0p = ctx.enter_context(tc.tile_pool(name="z0", bufs=1))
        zp = ctx.enter_context(tc.tile_pool(name="z", bufs=2))
        hp = ctx.enter_context(tc.tile_pool(name="h", bufs=2))
        dp = ctx.enter_context(tc.tile_pool(name="d", bufs=2))
        psum = ctx.enter_context(tc.tile_pool(name="ps", bufs=6, space="PSUM"))

        # resident weights / bias
        w0_sb = const.tile([P0, KB0, O], bf, tag="w0")
        nc.sync.dma_start(w0_sb, w0_d)
        w1_sb = const.tile([FK, F0, O], bf, tag="w1")
        nc.sync.dma_start(w1_sb, w1_d)
        w2_sb = const.tile([FK, F0, O], bf, tag="w2")
        nc.sync.dma_start(w2_sb, w2_d)
        b_sb = const.tile([FK, 6], f32, tag="bias")
        nc.sync.dma_start(b_sb, b_d)

        # output accumulators (j-group on partitions, batch on free)
        acc = [accp.tile([FK, BC], f32, tag=f"acc{i}", name=f"acc{i}") for i in range(4)]

        for rt in range(NRT):
            bs = slice(rt * BPT, (rt + 1) * BPT)
            xslab = x_d[rt]  # (F0, RT), contiguous

            # ---- layer-0 inputs: XR0[p=(dm,n),kb,r] = x[3kb+dm, r]; XT0[p] = x[n, r]
            xr0 = xr0p.tile([P0, KB0, RT], bf, tag="xr0")
            for dmi in range(G0):
                nc.sync.dma_start(
                    xr0[dmi * F0:(dmi + 1) * F0, :, :],
                    x0_d[rt, dmi][None, :, :].to_broadcast((F0, KB0, RT)))
            xt0 = xt0p.tile([P0, RT], bf, tag="xt0")
            nc.sync.dma_start(xt0, xslab[None, :, :].to_broadcast((G0, F0, RT)))

            # XR for layers 1/2: (FK, F0, RT) = x broadcast over 100 partitions
            xr = xrp.tile([FK, F0, RT], bf, tag="xr")
            nc.sync.dma_start(xr, xslab[None, :, :].to_broadcast((FK, F0, RT)))

            # ---- layer 0
            z0 = z0p.tile([P0, KB0, RT], bf, tag="z0")
            nc.gpsimd.tensor_tensor(
                z0, xr0, xt0[:, None, :].to_broadcast((P0, KB0, RT)), mult)
            ps0 = [psum.tile([FK, RT], f32, tag="ps", name=f"ps0_{rt}_{t}") for t in range(2)]
            for kb in range(KB0):
                st = kb == 0
                sp = kb == KB0 - 1
                for t in range(2):
                    nc.tensor.matmul(ps0[t], w0_sb[:, kb, t * FK:(t + 1) * FK],
                                     z0[:, kb, :], start=st, stop=sp)
            h1 = hp.tile([FK, RT], bf, tag="h")
            nc.scalar.activation(h1, ps0[0], relu, bias=b_sb[:, 0:1])
            d0 = dp.tile([FK, RT], bf, tag="d")
            nc.scalar.activation(d0, ps0[1], relu, bias=b_sb[:, 1:2])
            nc.vector.tensor_reduce(
                acc[0][:, bs], d0.rearrange("o (g f) -> o g f", f=D), X, add)

            # ---- layers 1, 2
            hprev = h1
            for li, (w_sb, bcol) in enumerate(((w1_sb, 2), (w2_sb, 4))):
                ps = [psum.tile([FK, RT], f32, tag="ps", name=f"psl{li}_{rt}_{t}") for t in range(2)]
                m0 = 0
                for g, zg in enumerate(ZGS):
                    zt = zp.tile([FK, ZGMAX, RT], bf, tag="z")
                    nc.vector.tensor_tensor(
                        zt[:, :zg, :], xr[:, m0:m0 + zg, :],
                        hprev[:, None, :].to_broadcast((FK, zg, RT)), mult)
                    for t in range(2):
                        for j in range(zg):
                            m = m0 + j
                            nc.tensor.matmul(ps[t], w_sb[:, m, t * FK:(t + 1) * FK],
                                             zt[:, j, :], start=(m == 0),
                                             stop=(m == F0 - 1))
                    m0 += zg
                if li == 0:
                    h2 = hp.tile([FK, RT], bf, tag="h")
                    nc.scalar.activation(h2, ps[0], relu, bias=b_sb[:, bcol:bcol + 1])
                    d1 = dp.tile([FK, RT], bf, tag="d")
                    nc.scalar.activation(d1, ps[1], relu, bias=b_sb[:, bcol + 1:bcol + 2])
                    nc.vector.tensor_reduce(
                        acc[1][:, bs], d1.rearrange("o (g f) -> o g f", f=D), X, add)
                    hprev = h2
                else:
                    for t in range(2):
                        d2 = dp.tile([FK, RT], bf, tag="d")
                        nc.scalar.activation(d2, ps[t], relu,
                                             bias=b_sb[:, bcol + t:bcol + t + 1])
                        nc.vector.tensor_reduce(
                            acc[2 + t][:, bs],
                            d2.rearrange("o (g f) -> o g f", f=D), X, add)

        for i in range(4):
            nc.sync.dma_start(out_d[i * FK:(i + 1) * FK, :], acc[i])


def _prep_weights(W0, W1, W2, b0, b1, b2):
    w0 = np.ascontiguousarray(
        W0.reshape(KB0, P0, O).transpose(1, 0, 2)).astype(BF16)
    w1 = np.ascontiguousarray(
        W1.reshape(F0, FK, O).transpose(1, 0, 2)).astype(BF16)
    w2 = np.ascontiguousarray(
        W2.reshape(F0, FK, O).transpose(1, 0, 2)).astype(BF16)
    bias = np.ascontiguousarray(
        np.stack([b0, b1, b2]).reshape(3, 2, FK).transpose(2, 0, 1).reshape(FK, 6)
    ).astype(np.float32)
    return w0, w1, w2, bias


def _prep_x_shard(x, c):
    xs = x[c * BC:(c + 1) * BC]                      # (BC, 39, 32)
    xt = xs.transpose(1, 0, 2).reshape(F0, ROWS)          # (39, 8192)
    x_t = np.ascontiguousarray(
        xt.reshape(F0, NRT, RT).transpose(1, 0, 2)).astype(BF16)
    x_perm = np.ascontiguousarray(
        xt.reshape(KB0, G0, NRT, RT).transpose(2, 1, 0, 3)).astype(BF16)
    return x_t, x_perm


def _build():
    if 'nc' in _cached:
        return _cached['nc']
    nc = bacc.Bacc("TRN2", target_bir_lowering=False, debug=False,
                   enable_asserts=False, num_devices=NCORES)
    ins = {
        'x_t': nc.dram_tensor("x_t", (NRT, F0, RT), mybir.dt.bfloat16,
                              kind="ExternalInput").ap(),
        'x0': nc.dram_tensor("x0", (NRT, G0, KB0, RT), mybir.dt.bfloat16,
                             kind="ExternalInput").ap(),
        'w0': nc.dram_tensor("w0", (P0, KB0, O), mybir.dt.bfloat16,
                             kind="ExternalInput").ap(),
        'w1': nc.dram_tensor("w1", (FK, F0, O), mybir.dt.bfloat16,
                             kind="ExternalInput").ap(),
        'w2': nc.dram_tensor("w2", (FK, F0, O), mybir.dt.bfloat16,
                             kind="ExternalInput").ap(),
        'bias': nc.dram_tensor("bias", (FK, 6), mybir.dt.float32,
                               kind="ExternalInput").ap(),
    }
    outs = {
        'out': nc.dram_tensor("out", (4 * FK, BC), mybir.dt.float32,
                              kind="ExternalOutput").ap(),
    }
    with tile.TileContext(nc, trace_sim=False) as tc:
        _emit(tc, outs, ins)
    nc.compile()
    _cached['nc'] = nc
    return nc


def kernel(x, W0, W1, W2, b0, b1, b2):
    nc = _build()
    w0, w1, w2, bias = _prep_weights(
        np.asarray(W0, np.float32), np.asarray(W1, np.float32),
        np.asarray(W2, np.float32), np.asarray(b0, np.float32),
        np.asarray(b1, np.float32), np.asarray(b2, np.float32))
    x = np.asarray(x, np.float32)
    in_maps = []
    for c in range(NCORES):
        x_t, x_perm = _prep_x_shard(x, c)
        in_maps.append({
            'x_t': x_t, 'x0': x_perm,
            'w0': w0, 'w1': w1, 'w2': w2, 'bias': bias,
        })
    res = bass_utils.run_bass_kernel_spmd(
        nc, in_maps, core_ids=list(range(NCORES)))
    out = np.empty((B, 4 * FK), np.float32)
    for c in range(NCORES):
        out[c * BC:(c + 1) * BC, :] = res.results[c]['out'].T
    return out
